# revision 1
# baseline (speedup 1.0000x reference)
"""DWT (db4-style, depthwise stride-2, reflect-pad) layer as a Trainium2
Bass/Tile kernel.

Math: for input x[B, T, C] and 8-tap filters lo/hi the reference computes a
reflect-pad-7, stride-2, depthwise cross-correlation cropped by 3 per side:

    out[b, t', c]     = sum_k lo[k] * xe[b, 2 t' + k, c]
    out[b, t', C + c] = sum_k hi[k] * xe[b, 2 t' + k, c]

with xe[u] = x[u - 1] for u in [1, T+1), xe[0] = x[1], xe[T+1] = x[T-2]
(after the crop only one reflected element is needed per side), and
t' in [0, T/2 - 2).

Device mapping (per core).  The binding resource on this part is DMA
DESCRIPTORS: the HWDGE generates ~50M descriptors/s (shared), SWDGE ~8.5
ns/desc serial on GpSimd, and descriptors below 2 KB also waste DMA-engine
cycles.  So both sides of the kernel are laid out for few, fat descriptors:

  - time on the SBUF partition axis one step per partition; ONE stationary
    matrix W[128, 122] holds BOTH filters as stride-2 bands (W[2m+k, m] =
    lo[k], W[2m+k, 61+m] = hi[k]).  One f32r matmul per pair of 61-output
    blocks (rhs [128, 512]) -- f32r is 4x faster than fp32 at free >= 256.
  - loads: the HOST pre-tiles xe = [x[1], x, x[T-2]] into supertile-major
    x_t[s, p, h*256] = xe[122*(16s+h) + p], so each SBUF partition line of
    a supertile is one CONTIGUOUS 16 KB run in DRAM: 128 descriptors per
    1.9 MB load, ~1.2K load descriptors per core total.
  - stores: outputs go to DRAM in q-major order lo_dev[m, q, (b,c)] with
    t' = 61q + m, so each store descriptor covers a supertile's worth of
    q per partition (8 KB): 61 descriptors per store, SWDGE (gpsimd).
    Outputs are bf16 (tolerance is 2e-2, bf16 adds ~2e-3) halving store
    bytes; lo/hi are separate tensors so PSUM [122, 512] is evacuated by
    ONE whole-tile DVE/Act copy (no interleave, no partition shifts).
  - the host un-permutes (transpose+reshape), upcasts, and concatenates.

Sharding: data-parallel over batch, 4 batches per core on 8 cores.
"""

import numpy as np

import concourse.bacc as bacc
import concourse.mybir as mybir
import concourse.tile as tile
from concourse.bass_utils import run_bass_kernel_spmd

F32 = mybir.dt.float32
F32R = mybir.dt.float32r
BF16 = mybir.dt.bfloat16

B, T, C = 32, 16384, 64
N_CORES = 8
BL = B // N_CORES   # 4 batches per core
NF = BL * C         # 256 floats = 1 KB per time step
M = 61              # outputs per block (2M+6 = 128-step window)
NOUT = T // 2 - 2   # 8190
NBLK = 134          # full blocks; NBLK*M = 8174
TAIL_T0 = NBLK * M  # 8174
TAIL_N = NOUT - TAIL_T0  # 16
H = 16              # blocks per load supertile / store group
NSUP = (NBLK + H - 1) // H  # 9: 8 full + 1 with 6 blocks
NQ = NBLK + 1       # q dim of the output tensors (tail block is q=134)


def _build_nc(store_dtype=BF16, mm_dtype=BF16):
    nc = bacc.Bacc("TRN2", target_bir_lowering=False, debug=False)
    x_d = nc.dram_tensor("x", [NSUP, 128, H * NF], mm_dtype,
                         kind="ExternalInput")
    xt_d = nc.dram_tensor("xtail", [38, NF], mm_dtype, kind="ExternalInput")
    w_d = nc.dram_tensor("w", [128, 122], mm_dtype, kind="ExternalInput")
    lo_d = nc.dram_tensor("lo", [M, NQ, NF], store_dtype, kind="ExternalOutput")
    hi_d = nc.dram_tensor("hi", [M, NQ, NF], store_dtype, kind="ExternalOutput")

    with tile.TileContext(nc) as tc:
        with (
            tc.tile_pool(name="wpool", bufs=1) as wpool,
            tc.tile_pool(name="xin", bufs=4) as xpool,
            tc.tile_pool(name="oout", bufs=6) as opool,
            tc.tile_pool(name="ps", bufs=7, space="PSUM") as pspool,
        ):
            # matmul inputs are bf16: the host pre-casts x and w, halving
            # load HBM traffic; bf16 matmuls run 1 cycle/row at any p-state
            w_t = wpool.tile([128, 122], mm_dtype)
            nc.sync.dma_start(out=w_t[:], in_=w_d[:])

            pair_ctr = 0
            for s in range(NSUP):
                q0 = H * s
                hs = min(H, NBLK - q0)  # 16, last is 6
                xt = xpool.tile([128, H * NF], mm_dtype, tag="xt")
                nc.sync.dma_start(out=xt[:, 0:hs * NF],
                                  in_=x_d[s, :, 0:hs * NF])

                st = opool.tile([122, H * NF], store_dtype, tag="st")
                for p in range(hs // 2):
                    ps = pspool.tile([122, 2 * NF], F32, tag="ps")
                    rhs = xt[:, 2 * p * NF:(2 * p + 2) * NF]
                    nc.tensor.matmul(out=ps[:], lhsT=w_t[:], rhs=rhs)
                    dst = st[:, 2 * p * NF:(2 * p + 2) * NF]
                    if pair_ctr % 2 == 0:
                        nc.vector.tensor_copy(out=dst, in_=ps[:])
                    else:
                        nc.scalar.copy(out=dst, in_=ps[:])
                    pair_ctr += 1

                # one fat-descriptor store per filter per supertile via SWDGE:
                # partition m's data for q in [q0, q0+hs) is one contiguous
                # hs*512B run in DRAM.  The SWDGE store stream is a ~120 GB/s
                # pipe (~4-engine pipeline fed by ring queue 0); supertile
                # granularity measured best (v7 2-supertile units starved
                # staging; v8 half-supertile units halved the stream rate).
                # HWDGE stores pin to ONE engine and block the issuer -- no.
                sv = st[:].rearrange("p (q w) -> p q w", w=NF)
                nc.gpsimd.dma_start(out=lo_d[:, q0:q0 + hs, :],
                                    in_=sv[0:M, 0:hs])
                nc.gpsimd.dma_start(out=hi_d[:, q0:q0 + hs, :],
                                    in_=sv[M:2 * M, 0:hs])

            # tail block: outputs 8174..8189 (16) stored at q=134
            xt_t = xpool.tile([38, NF], mm_dtype, tag="xtt", bufs=1)
            nc.sync.dma_start(out=xt_t[:], in_=xt_d[:])
            ps_t = pspool.tile([122, NF], F32, tag="pst", bufs=1)
            nc.tensor.matmul(out=ps_t[:], lhsT=w_t[0:38, :], rhs=xt_t[:])
            st_t = opool.tile([122, NF], store_dtype, tag="stt", bufs=1)
            nc.vector.tensor_copy(out=st_t[:], in_=ps_t[:])
            nc.gpsimd.dma_start(out=lo_d[0:TAIL_N, NBLK, :],
                                in_=st_t[0:TAIL_N])
            nc.gpsimd.dma_start(out=hi_d[0:TAIL_N, NBLK, :],
                                in_=st_t[M:M + TAIL_N])

    nc.compile()
    return nc


def _np_bf16():
    import ml_dtypes
    return ml_dtypes.bfloat16


def _build_w(dec_lo: np.ndarray, dec_hi: np.ndarray) -> np.ndarray:
    """Combined banded stationary matrix [128, 122]: cols 0:61 lo, 61:122 hi."""
    lo = np.asarray(dec_lo, np.float32)
    hi = np.asarray(dec_hi, np.float32)
    w = np.zeros((128, 122), np.float32)
    for m in range(M):
        for k in range(8):
            w[2 * m + k, m] = lo[k]
            w[2 * m + k, 61 + m] = hi[k]
    return w.astype(_np_bf16())


def _prep_core(x: np.ndarray, i: int) -> dict:
    """Host-side input prep for core i: supertile-tiled xe (bf16) + tail."""
    bf16 = _np_bf16()
    xc = np.ascontiguousarray(
        x[i * BL:(i + 1) * BL].transpose(1, 0, 2)).reshape(T, NF)
    xe = np.concatenate([xc[1:2], xc, xc[T - 2:T - 1]], axis=0).astype(bf16)
    # win[q, p, :] = xe[122q + p]
    win = np.lib.stride_tricks.as_strided(
        xe, shape=(NBLK, 128, NF),
        strides=(122 * xe.strides[0], xe.strides[0], xe.strides[1]))
    x_t = np.zeros((NSUP, 128, H * NF), bf16)
    for s in range(NSUP):
        hs = min(H, NBLK - H * s)
        x_t[s, :, 0:hs * NF] = (
            win[H * s:H * s + hs].transpose(1, 0, 2).reshape(128, hs * NF))
    x_tail = np.ascontiguousarray(xe[2 * TAIL_T0:2 * TAIL_T0 + 38])
    return {"x": x_t, "xtail": x_tail}


_NC_CACHE = {}


def _get_nc():
    key = "v6"
    if key not in _NC_CACHE:
        _NC_CACHE[key] = _build_nc()
    return _NC_CACHE[key]


def kernel(x: np.ndarray, dec_lo: np.ndarray, dec_hi: np.ndarray) -> np.ndarray:
    x = np.asarray(x, np.float32)
    assert x.shape == (B, T, C), x.shape
    nc = _get_nc()
    w = _build_w(dec_lo, dec_hi)
    in_maps = []
    for i in range(N_CORES):
        m = _prep_core(x, i)
        m["w"] = w
        in_maps.append(m)
    res = run_bass_kernel_spmd(nc, in_maps, core_ids=list(range(N_CORES)))
    out = np.empty((B, NOUT, 2 * C), np.float32)
    for i in range(N_CORES):
        # [M, NQ, NF] q-major -> t' = 61q + m ordered [NOUT, BL, C]
        lo = np.asarray(res.results[i]["lo"]).astype(np.float32)
        hi = np.asarray(res.results[i]["hi"]).astype(np.float32)
        lo = lo.transpose(1, 0, 2).reshape(NQ * M, BL, C)[:NOUT]
        hi = hi.transpose(1, 0, 2).reshape(NQ * M, BL, C)[:NOUT]
        out[i * BL:(i + 1) * BL] = np.concatenate(
            [lo, hi], axis=-1).transpose(1, 0, 2)
    return out



# revision 2
# speedup vs baseline: 1.1296x; 1.1296x over previous
"""DWT (db4-style, depthwise stride-2, reflect-pad) layer as a Trainium2
Bass/Tile kernel.

Math: for input x[B, T, C] and 8-tap filters lo/hi the reference computes a
reflect-pad-7, stride-2, depthwise cross-correlation cropped by 3 per side:

    out[b, t', c]     = sum_k lo[k] * xe[b, 2 t' + k, c]
    out[b, t', C + c] = sum_k hi[k] * xe[b, 2 t' + k, c]

with xe[u] = x[u - 1] for u in [1, T+1), xe[0] = x[1], xe[T+1] = x[T-2]
(after the crop only one reflected element is needed per side), and
t' in [0, T/2 - 2).

Device mapping (per core).  This kernel is HBM-bound: ~8.9 MB of loads and
~8.4 MB of stores per core against a ~358 GB/s per-NeuronCore HBM pipe, so
both directions are laid out for few, FAT DMA descriptors:

  - time on the SBUF partition axis one step per partition; ONE stationary
    matrix W[128, 122] holds BOTH filters as stride-2 bands (W[2m+k, m] =
    lo[k], W[2m+k, 61+m] = hi[k]).  One bf16 matmul per pair of 61-output
    blocks (rhs [128, 512]).
  - loads: the HOST pre-tiles xe = [x[1], x, x[T-2], 0-pad] into
    supertile-major x_t[s, p, h*256] = xe[122*(16s+h) + p], so each SBUF
    partition line of a supertile is one CONTIGUOUS 8 KB run in DRAM:
    128 descriptors per 1 MB load.
  - stores: ONE partition-major output tensor out[122, NQ*NF] (rows 0:61
    lo, 61:122 hi; per row q-major then (b,c)).  A supertile's outputs are
    a 2D contiguous slice out[:, q0*NF:(q0+hs)*NF], so each store is 122
    fat 8 KB descriptors (SWDGE/gpsimd) -- same shape as the loads.
    Outputs are bf16 (tolerance 2e-2, bf16 adds ~2e-3) halving store
    bytes; PSUM [122, 512] is evacuated by whole-tile DVE/Act copies.
  - the tail (outputs 8174..8189) is folded into a zero-padded 135th
    block: no special path, host drops t' >= 8190.
  - the host un-permutes (transpose+reshape), upcasts, and concatenates.

Sharding: data-parallel over batch, 4 batches per core on 8 cores.
"""

import numpy as np

import concourse.bacc as bacc
import concourse.mybir as mybir
import concourse.tile as tile
from concourse.bass_utils import run_bass_kernel_spmd

F32 = mybir.dt.float32
BF16 = mybir.dt.bfloat16

B, T, C = 32, 16384, 64
N_CORES = 8
BL = B // N_CORES   # 4 batches per core
NF = BL * C         # 256 floats = 512 B bf16 per time step
M = 61              # outputs per block (2M+6 = 128-step window)
NOUT = T // 2 - 2   # 8190
NBLK = 135          # blocks incl. zero-padded tail; NBLK*M = 8235 >= NOUT
H = 16              # blocks per load supertile / store group
NSUP = (NBLK + H - 1) // H  # 9: 8 full + 1 with 7 blocks
XE_LEN = 122 * (NBLK - 1) + 128  # 16476 padded xe length


def _build_nc(store_dtype=BF16, mm_dtype=BF16):
    nc = bacc.Bacc("TRN2", target_bir_lowering=False, debug=False)
    x_d = nc.dram_tensor("x", [NSUP, 128, H * NF], mm_dtype,
                         kind="ExternalInput")
    w_d = nc.dram_tensor("w", [128, 122], mm_dtype, kind="ExternalInput")
    out_d = nc.dram_tensor("out", [122, NBLK * NF], store_dtype,
                           kind="ExternalOutput")

    with tile.TileContext(nc) as tc:
        with (
            tc.tile_pool(name="wpool", bufs=1) as wpool,
            tc.tile_pool(name="xin", bufs=4) as xpool,
            tc.tile_pool(name="oout", bufs=6) as opool,
            tc.tile_pool(name="ps", bufs=8, space="PSUM") as pspool,
        ):
            # matmul inputs are bf16: the host pre-casts x and w, halving
            # load HBM traffic; bf16 matmuls run 1 cycle/row at any p-state
            w_t = wpool.tile([128, 122], mm_dtype)
            nc.sync.dma_start(out=w_t[:], in_=w_d[:])

            pair_ctr = 0
            for s in range(NSUP):
                q0 = H * s
                hs = min(H, NBLK - q0)  # 16, last is 7
                xt = xpool.tile([128, H * NF], mm_dtype, tag="xt")
                nc.sync.dma_start(out=xt[:, 0:hs * NF],
                                  in_=x_d[s, :, 0:hs * NF])

                st = opool.tile([122, H * NF], store_dtype, tag="st")
                for p in range(hs // 2):
                    ps = pspool.tile([122, 2 * NF], F32, tag="ps")
                    rhs = xt[:, 2 * p * NF:(2 * p + 2) * NF]
                    nc.tensor.matmul(out=ps[:], lhsT=w_t[:], rhs=rhs)
                    dst = st[:, 2 * p * NF:(2 * p + 2) * NF]
                    if pair_ctr % 2 == 0:
                        nc.vector.tensor_copy(out=dst, in_=ps[:])
                    else:
                        nc.scalar.copy(out=dst, in_=ps[:])
                    pair_ctr += 1
                if hs % 2:  # odd block count in the last supertile
                    ps = pspool.tile([122, 2 * NF], F32, tag="ps")
                    rhs = xt[:, (hs - 1) * NF:hs * NF]
                    nc.tensor.matmul(out=ps[:, 0:NF], lhsT=w_t[:], rhs=rhs)
                    dst = st[:, (hs - 1) * NF:hs * NF]
                    if pair_ctr % 2 == 0:
                        nc.vector.tensor_copy(out=dst, in_=ps[:, 0:NF])
                    else:
                        nc.scalar.copy(out=dst, in_=ps[:, 0:NF])
                    pair_ctr += 1

                # one store per supertile: 2D contiguous slice -> 122 fat
                # 8 KB descriptors on the SWDGE (gpsimd) queue, leaving the
                # sync HWDGE ring free for the loads.
                nc.gpsimd.dma_start(out=out_d[:, q0 * NF:(q0 + hs) * NF],
                                    in_=st[:, 0:hs * NF])

    nc.compile()
    return nc


def _np_bf16():
    import ml_dtypes
    return ml_dtypes.bfloat16


def _build_w(dec_lo: np.ndarray, dec_hi: np.ndarray) -> np.ndarray:
    """Combined banded stationary matrix [128, 122]: cols 0:61 lo, 61:122 hi."""
    lo = np.asarray(dec_lo, np.float32)
    hi = np.asarray(dec_hi, np.float32)
    w = np.zeros((128, 122), np.float32)
    for m in range(M):
        for k in range(8):
            w[2 * m + k, m] = lo[k]
            w[2 * m + k, 61 + m] = hi[k]
    return w.astype(_np_bf16())


def _prep_core(x: np.ndarray, i: int) -> dict:
    """Host-side input prep for core i: supertile-tiled padded xe (bf16)."""
    bf16 = _np_bf16()
    xc = np.ascontiguousarray(
        x[i * BL:(i + 1) * BL].transpose(1, 0, 2)).reshape(T, NF)
    xe = np.zeros((XE_LEN, NF), bf16)
    xe[0] = xc[1]
    xe[1:T + 1] = xc
    xe[T + 1] = xc[T - 2]
    # win[q, p, :] = xe[122q + p]
    win = np.lib.stride_tricks.as_strided(
        xe, shape=(NBLK, 128, NF),
        strides=(122 * xe.strides[0], xe.strides[0], xe.strides[1]))
    x_t = np.zeros((NSUP, 128, H * NF), bf16)
    for s in range(NSUP):
        hs = min(H, NBLK - H * s)
        x_t[s, :, 0:hs * NF] = (
            win[H * s:H * s + hs].transpose(1, 0, 2).reshape(128, hs * NF))
    return {"x": x_t}


_NC_CACHE = {}


def _get_nc():
    key = "v7"
    if key not in _NC_CACHE:
        _NC_CACHE[key] = _build_nc()
    return _NC_CACHE[key]


def kernel(x: np.ndarray, dec_lo: np.ndarray, dec_hi: np.ndarray) -> np.ndarray:
    x = np.asarray(x, np.float32)
    assert x.shape == (B, T, C), x.shape
    nc = _get_nc()
    w = _build_w(dec_lo, dec_hi)
    in_maps = []
    for i in range(N_CORES):
        m = _prep_core(x, i)
        m["w"] = w
        in_maps.append(m)
    res = run_bass_kernel_spmd(nc, in_maps, core_ids=list(range(N_CORES)))
    out = np.empty((B, NOUT, 2 * C), np.float32)
    for i in range(N_CORES):
        # [122, NBLK*NF] row-major -> t' = 61q + m ordered [NOUT, BL, C]
        o = np.asarray(res.results[i]["out"]).astype(np.float32)
        o = o.reshape(122, NBLK, BL, C)
        lo = o[0:M].transpose(1, 0, 2, 3).reshape(NBLK * M, BL, C)[:NOUT]
        hi = o[M:2 * M].transpose(1, 0, 2, 3).reshape(NBLK * M, BL, C)[:NOUT]
        out[i * BL:(i + 1) * BL] = np.concatenate(
            [lo, hi], axis=-1).transpose(1, 0, 2)
    return out
